# revision 3
# baseline (speedup 1.0000x reference)
"""TRN2 Bass kernel for nn_Attention (cross-attention, Tq=2, Tk=5, B=16384, D=512).

Math reformulation (exact):
    logits = h . k~,  k~ = e @ W_qk,  W_qk = Wk @ Wq^T
    att = softmax(logits)
    out = h@Wd1 + ctx@W_vd,   ctx = att @ e,   W_vd = Wv @ Wd2
This removes the q and v projections entirely.

Sharding: pure data parallel over batch, 2048 per core x 8 cores.
On-chip compute in fp16 (PSUM accumulation fp32); final out fp32.
"""

import numpy as np

import concourse.bass as bass
import concourse.mybir as mybir
import concourse.tile as tile
from concourse import bacc
from concourse.bass_utils import run_bass_kernel_spmd
from concourse.masks import make_identity

F32 = mybir.dt.float32
F16 = mybir.dt.float16
MUL = mybir.AluOpType.mult
ADD = mybir.AluOpType.add
BYP = mybir.AluOpType.bypass

TQ, TK, B, D = 2, 5, 16384, 512
NCORES = 8
BL = B // NCORES          # 2048 batch per core
P = 128                   # partition tile
NT = BL // P              # 16 batch tiles per core
DC = D // P               # 4 contraction chunks

_CACHED = {}


def build(reps=1):
    nc = bacc.Bacc("TRN2", target_bir_lowering=False, debug=False)

    h_d = nc.dram_tensor("h", [TQ, BL, D], F32, kind="ExternalInput")
    e_d = nc.dram_tensor("enc", [TK, BL, D], F32, kind="ExternalInput")
    wq_d = nc.dram_tensor("Wq", [D, D], F32, kind="ExternalInput")
    wk_d = nc.dram_tensor("Wk", [D, D], F32, kind="ExternalInput")
    wv_d = nc.dram_tensor("Wv", [D, D], F32, kind="ExternalInput")
    wd_d = nc.dram_tensor("Wdown", [2 * D, D], F32, kind="ExternalInput")
    o_d = nc.dram_tensor("out", [TQ, BL, D], F32, kind="ExternalOutput")

    h_r = h_d.ap().rearrange("i b d -> b i d")
    e_r = e_d.ap().rearrange("j b d -> b j d")
    o_r = o_d.ap().rearrange("i b d -> b i d")

    with tile.TileContext(nc) as tc:
        with (
            tc.tile_pool(name="wgt", bufs=1) as wgt,
            tc.tile_pool(name="pre", bufs=1) as pre,
            tc.tile_pool(name="io", bufs=3) as io,
            tc.tile_pool(name="work", bufs=3) as work,
            tc.tile_pool(name="small", bufs=3) as small,
            tc.tile_pool(name="ps", bufs=2, space="PSUM") as ps,       # [P,8,P] 2 banks x2
            tc.tile_pool(name="psk", bufs=1, space="PSUM") as psk,     # [P,2,512] 2 banks
            tc.tile_pool(name="pso", bufs=1, space="PSUM") as pso,     # [P,2,512] 2 banks
        ):
            ident = wgt.tile([P, P], F16)
            make_identity(nc, ident)

            # ---- load weights (cast fp32 -> fp16 during DMA) ----
            wq16 = pre.tile([P, DC, D], F16, tag="wq16")
            wk16 = pre.tile([P, DC, D], F16, tag="wk16")
            wv16 = pre.tile([P, DC, D], F16, tag="wv16")
            wd1 = wgt.tile([P, DC, D], F16, tag="wd1")
            wd2 = wgt.tile([P, DC, D], F16, tag="wd2")
            nc.gpsimd.dma_start(out=wq16, in_=wq_d.ap().rearrange("(c p) n -> p c n", p=P))
            nc.gpsimd.dma_start(out=wk16, in_=wk_d.ap().rearrange("(c p) n -> p c n", p=P))
            nc.gpsimd.dma_start(out=wv16, in_=wv_d.ap().rearrange("(c p) n -> p c n", p=P))
            nc.gpsimd.dma_start(out=wd1, in_=wd_d.ap()[:D].rearrange("(c p) n -> p c n", p=P))
            nc.gpsimd.dma_start(out=wd2, in_=wd_d.ap()[D:].rearrange("(c p) n -> p c n", p=P))

            # ---- transpose Wq, Wk, Wv via identity-matmul (2 waves of 8 blocks) ----
            def transpose_weight(w16, name):
                wT = pre.tile([P, DC, D], F16, tag=name, name=name)
                for w in range(2):
                    pt = ps.tile([P, 8, P], F32, tag="pt", name=f"pt_{name}{w}")
                    for gg in range(2):
                        g = w * 2 + gg
                        for a in range(DC):
                            nc.tensor.matmul(
                                pt[:, gg * 4 + a, :], w16[:, a, g * P:(g + 1) * P],
                                ident, start=True, stop=True)
                    # pt free = (g, a, b) -> wT[:, g, a*P:+P]
                    nc.scalar.copy(
                        wT[:, w * 2:w * 2 + 2, :],
                        pt.rearrange("p (g a) b -> p g (a b)", g=2))
                return wT

            wqT = transpose_weight(wq16, "wqT")
            wkT = transpose_weight(wk16, "wkT")
            wvT = transpose_weight(wv16, "wvT")

            # ---- W_qk = Wk @ Wq^T ;  W_vd = Wv @ Wd2 ----
            wqk = wgt.tile([P, DC, D], F16, tag="wqk")
            wvd = wgt.tile([P, DC, D], F16, tag="wvd")
            for nm, (lhsT, rhs, dst) in {
                "q": (wkT, wqT, wqk), "v": (wvT, wd2, wvd)
            }.items():
                for w in range(2):
                    acc = psk.tile([P, 2, D], F32, tag="pk", name=f"pk_{nm}{w}")
                    for aa in range(2):
                        ach = w * 2 + aa
                        for g in range(DC):
                            nc.tensor.matmul(
                                acc[:, aa, :], lhsT[:, g, ach * P:(ach + 1) * P],
                                rhs[:, g, :], start=(g == 0), stop=(g == DC - 1))
                    nc.scalar.copy(dst[:, w * 2:w * 2 + 2, :], acc)

            # ---- main loop over batch tiles ----
            import contextlib
            loop_cm = tc.For_i(0, reps, 1) if reps > 1 else contextlib.nullcontext()
            with loop_cm:
                for t in range(NT):
                    b0 = t * P
                    bsl = slice(b0, b0 + P)

                    hn = io.tile([P, TQ, D], F16, tag="hn")
                    en = io.tile([P, TK, D], F16, tag="en")
                    nc.gpsimd.dma_start(out=hn, in_=h_r[bsl])
                    nc.gpsimd.dma_start(out=en, in_=e_r[bsl])

                    # --- transposes: h (8 blocks, 1 wave), enc (20 blocks, 3 waves) ---
                    hT = work.tile([P, DC, TQ, P], F16, tag="hT")
                    pth = ps.tile([P, 8, P], F32, tag="pt", name="pth")
                    for i in range(TQ):
                        for c in range(DC):
                            nc.tensor.matmul(
                                pth[:, i * DC + c, :], hn[:, i, c * P:(c + 1) * P],
                                ident, start=True, stop=True)
                    # pth free = (i, c, b) -> hT[:, c, i, :]
                    nc.scalar.copy(hT, pth.rearrange("p (i c) b -> p c i b", i=TQ))

                    eT = work.tile([P, DC, TK, P], F16, tag="eT")
                    for w in range(3):
                        js = (0, 2, 4)[w]
                        jn = (2, 2, 1)[w]
                        pte = ps.tile([P, 8, P], F32, tag="pt", name=f"pte{w}")
                        for jj in range(jn):
                            for c in range(DC):
                                nc.tensor.matmul(
                                    pte[:, jj * DC + c, :],
                                    en[:, js + jj, c * P:(c + 1) * P],
                                    ident, start=True, stop=True)
                        nc.scalar.copy(
                            eT[:, :, js:js + jn, :],
                            pte[:, 0:jn * DC, :].rearrange("p (j c) b -> p c j b", j=jn))

                    # --- k~ = enc @ W_qk (activation-stationary, natural out) ---
                    kn = work.tile([P, TK, D], F16, tag="kn")
                    for w in range(3):
                        js = (0, 2, 4)[w]
                        jn = (2, 2, 1)[w]
                        acc = psk.tile([P, 2, D], F32, tag="pk", name=f"pkk{w}")
                        for jj in range(jn):
                            for c in range(DC):
                                nc.tensor.matmul(
                                    acc[:, jj, :], eT[:, c, js + jj, :], wqk[:, c, :],
                                    start=(c == 0), stop=(c == DC - 1))
                        nc.scalar.copy(kn[:, js:js + jn, :], acc[:, 0:jn, :])

                    # --- logits[b, i, j] = h_i . k~_j  (STT bypass-mult + accum) ---
                    logits = small.tile([P, TQ, TK], F32, tag="logits")
                    pdump = small.tile([P, 1], F16, tag="pdump")
                    for i in range(TQ):
                        for j in range(TK):
                            nc.vector.scalar_tensor_tensor(
                                out=pdump.broadcast_to([P, D]),
                                in0=hn[:, i, :], scalar=1.0, in1=kn[:, j, :],
                                op0=BYP, op1=MUL,
                                accum_out=logits[:, i, j:j + 1])

                    # --- softmax over j ---
                    nmx = small.tile([P, TQ], F32, tag="nmx")
                    pr = small.tile([P, TQ, TK], F32, tag="pr")
                    sm = small.tile([P, TQ], F32, tag="sm")
                    rs = small.tile([P, TQ], F32, tag="rs")
                    attw = small.tile([P, TQ, TK], F32, tag="attw")
                    nc.vector.tensor_reduce(
                        out=nmx, in_=logits, axis=mybir.AxisListType.X,
                        op=mybir.AluOpType.max, negate=True)
                    for i in range(TQ):
                        nc.scalar.activation(
                            out=pr[:, i, :], in_=logits[:, i, :],
                            func=mybir.ActivationFunctionType.Exp,
                            bias=nmx[:, i:i + 1],
                            accum_out=sm[:, i:i + 1])
                    nc.vector.reciprocal(rs, sm)
                    for i in range(TQ):
                        nc.vector.tensor_scalar_mul(attw[:, i, :], pr[:, i, :], rs[:, i:i + 1])

                    # --- ctx_i = sum_j attw[i,j] * e_j  (STT mult-add chain, fp16) ---
                    ctx = work.tile([P, TQ, D], F16, tag="ctx")
                    for i in range(TQ):
                        nc.vector.tensor_scalar_mul(ctx[:, i, :], en[:, 0, :], attw[:, i, 0:1])
                        for j in range(1, TK):
                            nc.vector.scalar_tensor_tensor(
                                out=ctx[:, i, :], in0=en[:, j, :],
                                scalar=attw[:, i, j:j + 1], in1=ctx[:, i, :],
                                op0=MUL, op1=ADD)

                    # --- transpose ctx (8 blocks, 1 wave) ---
                    cT = work.tile([P, DC, TQ, P], F16, tag="cT")
                    ptc = ps.tile([P, 8, P], F32, tag="pt", name="ptc")
                    for i in range(TQ):
                        for c in range(DC):
                            nc.tensor.matmul(
                                ptc[:, i * DC + c, :], ctx[:, i, c * P:(c + 1) * P],
                                ident, start=True, stop=True)
                    nc.vector.tensor_copy(cT, ptc.rearrange("p (i c) b -> p c i b", i=TQ))

                    # --- out_i = h_i @ Wd1 + ctx_i @ W_vd ---
                    ob = io.tile([P, TQ, D], F32, tag="ob")
                    po = pso.tile([P, TQ, D], F32, tag="po")
                    for i in range(TQ):
                        for c in range(DC):
                            nc.tensor.matmul(po[:, i, :], hT[:, c, i, :], wd1[:, c, :],
                                             start=(c == 0), stop=False)
                        for c in range(DC):
                            nc.tensor.matmul(po[:, i, :], cT[:, c, i, :], wvd[:, c, :],
                                             start=False, stop=(c == DC - 1))
                    nc.vector.tensor_copy(ob, po)
                    nc.sync.dma_start(out=o_r[bsl], in_=ob)

    nc.compile()
    return nc


def kernel(h, enc_out, Wq, Wk, Wv, Wdown, _trace=False):
    h = np.ascontiguousarray(h, dtype=np.float32)
    enc_out = np.ascontiguousarray(enc_out, dtype=np.float32)
    Wq = np.ascontiguousarray(Wq, dtype=np.float32)
    Wk = np.ascontiguousarray(Wk, dtype=np.float32)
    Wv = np.ascontiguousarray(Wv, dtype=np.float32)
    Wdown = np.ascontiguousarray(Wdown, dtype=np.float32)

    if "nc" not in _CACHED:
        _CACHED["nc"] = build()
    nc = _CACHED["nc"]

    in_maps = []
    for c in range(NCORES):
        sl = slice(c * BL, (c + 1) * BL)
        in_maps.append({
            "h": np.ascontiguousarray(h[:, sl]),
            "enc": np.ascontiguousarray(enc_out[:, sl]),
            "Wq": Wq, "Wk": Wk, "Wv": Wv, "Wdown": Wdown,
        })

    res = run_bass_kernel_spmd(nc, in_maps, list(range(NCORES)), trace=_trace)
    out = np.concatenate([r["out"] for r in res.results], axis=1)
    if _trace:
        kernel.last_result = res
    return out.astype(np.float32)


# revision 7
# speedup vs baseline: 1.0022x; 1.0022x over previous
"""TRN2 Bass kernel for nn_Attention (cross-attention, Tq=2, Tk=5, B=16384, D=512).

Math reformulation (exact):
    logits = h . k~,  k~ = e @ W_qk,  W_qk = Wk @ Wq^T
    att = softmax(logits)
    out = h@Wd1 + ctx@W_vd,   ctx = att @ e,   W_vd = Wv @ Wd2
This removes the q and v projections entirely.

Sharding: pure data parallel over batch, 2048 per core x 8 cores.
On-chip compute in fp16 (PSUM accumulation fp32); final out fp32.
"""

import numpy as np

import concourse.bass as bass
import concourse.mybir as mybir
import concourse.tile as tile
from concourse import bacc
from concourse.bass_utils import run_bass_kernel_spmd
from concourse.masks import make_identity

F32 = mybir.dt.float32
F16 = mybir.dt.float16
MUL = mybir.AluOpType.mult
ADD = mybir.AluOpType.add
BYP = mybir.AluOpType.bypass

TQ, TK, B, D = 2, 5, 16384, 512
NCORES = 8
BL = B // NCORES          # 2048 batch per core
P = 128                   # partition tile
NT = BL // P              # 16 batch tiles per core
DC = D // P               # 4 contraction chunks

_CACHED = {}


def build(reps=1, skip=()):
    nc = bacc.Bacc("TRN2", target_bir_lowering=False, debug=False)

    h_d = nc.dram_tensor("h", [BL, TQ, D], F32, kind="ExternalInput")
    e_d = nc.dram_tensor("enc", [BL, TK, D], F32, kind="ExternalInput")
    wq_d = nc.dram_tensor("Wq", [D, D], F32, kind="ExternalInput")
    wk_d = nc.dram_tensor("Wk", [D, D], F32, kind="ExternalInput")
    wv_d = nc.dram_tensor("Wv", [D, D], F32, kind="ExternalInput")
    wd_d = nc.dram_tensor("Wdown", [2 * D, D], F32, kind="ExternalInput")
    o_d = nc.dram_tensor("out", [BL, TQ, D], F32, kind="ExternalOutput")

    h_r = h_d.ap()
    e_r = e_d.ap()
    o_r = o_d.ap()

    with tile.TileContext(nc) as tc:
        with (
            tc.tile_pool(name="wgt", bufs=1) as wgt,
            tc.tile_pool(name="pre", bufs=1) as pre,
            tc.tile_pool(name="io", bufs=3) as io,
            tc.tile_pool(name="work", bufs=3) as work,
            tc.tile_pool(name="small", bufs=3) as small,
            tc.tile_pool(name="ps", bufs=2, space="PSUM") as ps,       # [P,8,P] 2 banks x2
            tc.tile_pool(name="psk", bufs=1, space="PSUM") as psk,     # [P,2,512] 2 banks
            tc.tile_pool(name="pso", bufs=1, space="PSUM") as pso,     # [P,2,512] 2 banks
        ):
            ident = wgt.tile([P, P], F16)
            make_identity(nc, ident)

            # ---- load weights (cast fp32 -> fp16 during DMA) ----
            wq16 = pre.tile([P, DC, D], F16, tag="wq16")
            wk16 = pre.tile([P, DC, D], F16, tag="wk16")
            wv16 = pre.tile([P, DC, D], F16, tag="wv16")
            wd1 = wgt.tile([P, DC, D], F16, tag="wd1")
            wd2 = wgt.tile([P, DC, D], F16, tag="wd2")
            nc.gpsimd.dma_start(out=wq16, in_=wq_d.ap().rearrange("(c p) n -> p c n", p=P))
            nc.gpsimd.dma_start(out=wk16, in_=wk_d.ap().rearrange("(c p) n -> p c n", p=P))
            nc.gpsimd.dma_start(out=wv16, in_=wv_d.ap().rearrange("(c p) n -> p c n", p=P))
            nc.gpsimd.dma_start(out=wd1, in_=wd_d.ap()[:D].rearrange("(c p) n -> p c n", p=P))
            nc.gpsimd.dma_start(out=wd2, in_=wd_d.ap()[D:].rearrange("(c p) n -> p c n", p=P))

            # ---- transpose Wq, Wk, Wv via identity-matmul (2 waves of 8 blocks) ----
            def transpose_weight(w16, name):
                wT = pre.tile([P, DC, D], F16, tag=name, name=name)
                for w in range(2):
                    pt = ps.tile([P, 8, P], F32, tag="pt", name=f"pt_{name}{w}")
                    for gg in range(2):
                        g = w * 2 + gg
                        for a in range(DC):
                            nc.tensor.matmul(
                                pt[:, gg * 4 + a, :], w16[:, a, g * P:(g + 1) * P],
                                ident, start=True, stop=True)
                    # pt free = (g, a, b) -> wT[:, g, a*P:+P]
                    nc.scalar.copy(
                        wT[:, w * 2:w * 2 + 2, :],
                        pt.rearrange("p (g a) b -> p g (a b)", g=2))
                return wT

            wqT = transpose_weight(wq16, "wqT")
            wkT = transpose_weight(wk16, "wkT")
            wvT = transpose_weight(wv16, "wvT")

            # ---- W_qk = Wk @ Wq^T ;  W_vd = Wv @ Wd2 ----
            wqk = wgt.tile([P, DC, D], F16, tag="wqk")
            wvd = wgt.tile([P, DC, D], F16, tag="wvd")
            for nm, (lhsT, rhs, dst) in {
                "q": (wkT, wqT, wqk), "v": (wvT, wd2, wvd)
            }.items():
                for w in range(2):
                    acc = psk.tile([P, 2, D], F32, tag="pk", name=f"pk_{nm}{w}")
                    for aa in range(2):
                        ach = w * 2 + aa
                        for g in range(DC):
                            nc.tensor.matmul(
                                acc[:, aa, :], lhsT[:, g, ach * P:(ach + 1) * P],
                                rhs[:, g, :], start=(g == 0), stop=(g == DC - 1))
                    nc.scalar.copy(dst[:, w * 2:w * 2 + 2, :], acc)

            # ---- main loop over batch tiles ----
            pre_hn, pre_en = [], []
            if "dma" in skip:
                for t in range(NT):
                    phn = pre.tile([P, TQ, D], F16, tag=f"phn{t}", name=f"phn{t}")
                    pen = pre.tile([P, TK, D], F16, tag=f"pen{t}", name=f"pen{t}")
                    nc.gpsimd.dma_start(out=phn, in_=h_r[t * P:(t + 1) * P])
                    nc.gpsimd.dma_start(out=pen, in_=e_r[t * P:(t + 1) * P])
                    pre_hn.append(phn)
                    pre_en.append(pen)
            import contextlib
            loop_cm = tc.For_i(0, reps, 1) if reps > 1 else contextlib.nullcontext()
            with loop_cm:
                for t in range(NT):
                    b0 = t * P
                    bsl = slice(b0, b0 + P)

                    if "dma" in skip:
                        hn = pre_hn[t]
                        en = pre_en[t]
                    else:
                        hn = io.tile([P, TQ, D], F16, tag="hn")
                        en = io.tile([P, TK, D], F16, tag="en")
                        nc.gpsimd.dma_start(out=hn, in_=h_r[bsl])
                        nc.gpsimd.dma_start(out=en, in_=e_r[bsl])

                    # --- transposes: h (8 blocks, 1 wave), enc (20 blocks, 3 waves) ---
                    hT = work.tile([P, DC, TQ, P], F16, tag="hT")
                    pth = ps.tile([P, 8, P], F32, tag="pt", name="pth")
                    for i in range(TQ):
                        for c in range(DC):
                            nc.tensor.matmul(
                                pth[:, i * DC + c, :], hn[:, i, c * P:(c + 1) * P],
                                ident, start=True, stop=True)
                    # pth free = (i, c, b) -> hT[:, c, i, :]
                    nc.scalar.copy(hT, pth.rearrange("p (i c) b -> p c i b", i=TQ))

                    eT = work.tile([P, DC, TK, P], F16, tag="eT")
                    for w in range(3):
                        js = (0, 2, 4)[w]
                        jn = (2, 2, 1)[w]
                        pte = ps.tile([P, 8, P], F32, tag="pt", name=f"pte{w}")
                        for jj in range(jn):
                            for c in range(DC):
                                nc.tensor.matmul(
                                    pte[:, jj * DC + c, :],
                                    en[:, js + jj, c * P:(c + 1) * P],
                                    ident, start=True, stop=True)
                        nc.scalar.copy(
                            eT[:, :, js:js + jn, :],
                            pte[:, 0:jn * DC, :].rearrange("p (j c) b -> p c j b", j=jn))

                    # --- k~ = enc @ W_qk (activation-stationary, natural out) ---
                    kn = work.tile([P, TK, D], F16, tag="kn")
                    for w in range(3):
                        js = (0, 2, 4)[w]
                        jn = (2, 2, 1)[w]
                        acc = psk.tile([P, 2, D], F32, tag="pk", name=f"pkk{w}")
                        for jj in range(jn):
                            for c in range(DC):
                                nc.tensor.matmul(
                                    acc[:, jj, :], eT[:, c, js + jj, :], wqk[:, c, :],
                                    start=(c == 0), stop=(c == DC - 1))
                        nc.scalar.copy(kn[:, js:js + jn, :], acc[:, 0:jn, :])

                    # --- logits[b, i, j] = h_i . k~_j  (STT bypass-mult + accum) ---
                    if "attn" in skip:
                        ctx = work.tile([P, TQ, D], F16, tag="ctx")
                        nc.vector.tensor_copy(ctx, kn[:, 0:2, :])
                    logits = small.tile([P, TQ, TK], F32, tag="logits")
                    pdump = small.tile([P, 1], F16, tag="pdump")
                    for i in range(TQ if "attn" not in skip else 0):
                        for j in range(TK):
                            nc.vector.scalar_tensor_tensor(
                                out=pdump.broadcast_to([P, D]),
                                in0=hn[:, i, :], scalar=1.0, in1=kn[:, j, :],
                                op0=BYP, op1=MUL,
                                accum_out=logits[:, i, j:j + 1])

                    # --- softmax over j ---
                    nmx = small.tile([P, TQ], F32, tag="nmx")
                    pr = small.tile([P, TQ, TK], F32, tag="pr")
                    sm = small.tile([P, TQ], F32, tag="sm")
                    rs = small.tile([P, TQ], F32, tag="rs")
                    attw = small.tile([P, TQ, TK], F32, tag="attw")
                    if "attn" not in skip:
                      nc.vector.tensor_reduce(
                        out=nmx, in_=logits, axis=mybir.AxisListType.X,
                        op=mybir.AluOpType.max, negate=True)
                      for i in range(TQ):
                        nc.scalar.activation(
                            out=pr[:, i, :], in_=logits[:, i, :],
                            func=mybir.ActivationFunctionType.Exp,
                            bias=nmx[:, i:i + 1],
                            accum_out=sm[:, i:i + 1])
                      nc.vector.reciprocal(rs, sm)
                      for i in range(TQ):
                        nc.vector.tensor_scalar_mul(attw[:, i, :], pr[:, i, :], rs[:, i:i + 1])

                    # --- ctx_i = sum_j attw[i,j] * e_j  (STT mult-add chain, fp16) ---
                    if "attn" not in skip:
                      ctx = work.tile([P, TQ, D], F16, tag="ctx")
                      for i in range(TQ):
                        nc.vector.tensor_scalar_mul(ctx[:, i, :], en[:, 0, :], attw[:, i, 0:1])
                        for j in range(1, TK):
                            nc.vector.scalar_tensor_tensor(
                                out=ctx[:, i, :], in0=en[:, j, :],
                                scalar=attw[:, i, j:j + 1], in1=ctx[:, i, :],
                                op0=MUL, op1=ADD)

                    # --- transpose ctx (8 blocks, 1 wave) ---
                    cT = work.tile([P, DC, TQ, P], F16, tag="cT")
                    ptc = ps.tile([P, 8, P], F32, tag="pt", name="ptc")
                    for i in range(TQ):
                        for c in range(DC):
                            nc.tensor.matmul(
                                ptc[:, i * DC + c, :], ctx[:, i, c * P:(c + 1) * P],
                                ident, start=True, stop=True)
                    nc.vector.tensor_copy(cT, ptc.rearrange("p (i c) b -> p c i b", i=TQ))

                    # --- out_i = h_i @ Wd1 + ctx_i @ W_vd ---
                    ob = io.tile([P, TQ, D], F32, tag="ob")
                    po = pso.tile([P, TQ, D], F32, tag="po")
                    for i in range(TQ):
                        for c in range(DC):
                            nc.tensor.matmul(po[:, i, :], hT[:, c, i, :], wd1[:, c, :],
                                             start=(c == 0), stop=False)
                        for c in range(DC):
                            nc.tensor.matmul(po[:, i, :], cT[:, c, i, :], wvd[:, c, :],
                                             start=False, stop=(c == DC - 1))
                    nc.vector.tensor_copy(ob, po)
                    nc.gpsimd.dma_start(out=o_r[bsl], in_=ob)

    nc.compile()
    return nc


def kernel(h, enc_out, Wq, Wk, Wv, Wdown, _trace=False):
    h = np.ascontiguousarray(h, dtype=np.float32)
    enc_out = np.ascontiguousarray(enc_out, dtype=np.float32)
    Wq = np.ascontiguousarray(Wq, dtype=np.float32)
    Wk = np.ascontiguousarray(Wk, dtype=np.float32)
    Wv = np.ascontiguousarray(Wv, dtype=np.float32)
    Wdown = np.ascontiguousarray(Wdown, dtype=np.float32)

    if "nc" not in _CACHED:
        _CACHED["nc"] = build()
    nc = _CACHED["nc"]

    h_bm = np.ascontiguousarray(h.transpose(1, 0, 2))        # [B, TQ, D]
    e_bm = np.ascontiguousarray(enc_out.transpose(1, 0, 2))   # [B, TK, D]
    in_maps = []
    for c in range(NCORES):
        sl = slice(c * BL, (c + 1) * BL)
        in_maps.append({
            "h": h_bm[sl],
            "enc": e_bm[sl],
            "Wq": Wq, "Wk": Wk, "Wv": Wv, "Wdown": Wdown,
        })

    res = run_bass_kernel_spmd(nc, in_maps, list(range(NCORES)), trace=_trace)
    out_bm = np.concatenate([r["out"] for r in res.results], axis=0)  # [B, TQ, D]
    out = np.ascontiguousarray(out_bm.transpose(1, 0, 2))
    if _trace:
        kernel.last_result = res
    return out.astype(np.float32)


# revision 8
# speedup vs baseline: 1.2172x; 1.2145x over previous
"""TRN2 Bass kernel for nn_Attention (cross-attention, Tq=2, Tk=5, B=16384, D=512).

Math reformulation (exact):
    logits = h . k~,  k~ = e @ W_qk,  W_qk = Wk @ Wq^T
    att = softmax(logits)
    out = h@Wd1 + ctx@W_vd,   ctx = att @ e,   W_vd = Wv @ Wd2
This removes the q and v projections entirely.

Sharding: pure data parallel over batch, 2048 per core x 8 cores.
Host marshals inputs/outputs to batch-major [B, T, D] for contiguous DMA.
On-chip compute in fp16 (PSUM accumulation fp32); final out fp32.
Main loop is software-pipelined (front: loads/transposes/k~; back: attention/out)
with a lag of 2 batch tiles so PE and DVE streams interleave across tiles.
"""

import contextlib

import numpy as np

import concourse.bass as bass
import concourse.mybir as mybir
import concourse.tile as tile
from concourse import bacc
from concourse.bass_utils import run_bass_kernel_spmd
from concourse.masks import make_identity

F32 = mybir.dt.float32
F16 = mybir.dt.float16
MUL = mybir.AluOpType.mult
ADD = mybir.AluOpType.add
BYP = mybir.AluOpType.bypass

TQ, TK, B, D = 2, 5, 16384, 512
NCORES = 8
BL = B // NCORES          # 2048 batch per core
P = 128                   # partition tile
NT = BL // P              # 16 batch tiles per core
DC = D // P               # 4 contraction chunks
LAG = 2                   # software-pipeline depth (front of t  ||  back of t-LAG)

_CACHED = {}


def build(reps=1, skip=()):
    nc = bacc.Bacc("TRN2", target_bir_lowering=False, debug=False)

    h_d = nc.dram_tensor("h", [BL, TQ, D], F32, kind="ExternalInput")
    e_d = nc.dram_tensor("enc", [BL, TK, D], F32, kind="ExternalInput")
    wq_d = nc.dram_tensor("Wq", [D, D], F32, kind="ExternalInput")
    wk_d = nc.dram_tensor("Wk", [D, D], F32, kind="ExternalInput")
    wv_d = nc.dram_tensor("Wv", [D, D], F32, kind="ExternalInput")
    wd_d = nc.dram_tensor("Wdown", [2 * D, D], F32, kind="ExternalInput")
    o_d = nc.dram_tensor("out", [BL, TQ, D], F32, kind="ExternalOutput")

    h_r = h_d.ap()
    e_r = e_d.ap()
    o_r = o_d.ap()

    with tile.TileContext(nc) as tc:
        with (
            tc.tile_pool(name="wgt", bufs=1) as wgt,
            tc.tile_pool(name="pre", bufs=1) as pre,
            tc.tile_pool(name="io", bufs=LAG + 2) as io,
            tc.tile_pool(name="work", bufs=LAG + 2) as work,
            tc.tile_pool(name="bwork", bufs=2) as bwork,
            tc.tile_pool(name="small", bufs=3) as small,
            tc.tile_pool(name="ps", bufs=2, space="PSUM") as ps,       # "pt": [P,8,P] 2bk x2
            tc.tile_pool(name="psk", bufs=1, space="PSUM") as psk,     # "pk": [P,2,512] 2bk
            tc.tile_pool(name="psb", bufs=1, space="PSUM") as psb,     # "bk": 2bk (ptc/po)
        ):
            ident = wgt.tile([P, P], F16)
            make_identity(nc, ident)

            # ---- load weights (cast fp32 -> fp16 during DMA) ----
            wq16 = pre.tile([P, DC, D], F16, tag="wq16")
            wk16 = pre.tile([P, DC, D], F16, tag="wk16")
            wv16 = pre.tile([P, DC, D], F16, tag="wv16")
            wd1 = wgt.tile([P, DC, D], F16, tag="wd1")
            wd2 = wgt.tile([P, DC, D], F16, tag="wd2")
            nc.gpsimd.dma_start(out=wq16, in_=wq_d.ap().rearrange("(c p) n -> p c n", p=P))
            nc.gpsimd.dma_start(out=wk16, in_=wk_d.ap().rearrange("(c p) n -> p c n", p=P))
            nc.gpsimd.dma_start(out=wv16, in_=wv_d.ap().rearrange("(c p) n -> p c n", p=P))
            nc.gpsimd.dma_start(out=wd1, in_=wd_d.ap()[:D].rearrange("(c p) n -> p c n", p=P))
            nc.gpsimd.dma_start(out=wd2, in_=wd_d.ap()[D:].rearrange("(c p) n -> p c n", p=P))

            # ---- transpose Wq, Wk, Wv via identity-matmul (2 waves of 8 blocks) ----
            def transpose_weight(w16, name):
                wT = pre.tile([P, DC, D], F16, tag=name, name=name)
                for w in range(2):
                    pt = ps.tile([P, 8, P], F32, tag="pt", name=f"pt_{name}{w}")
                    for gg in range(2):
                        g = w * 2 + gg
                        for a in range(DC):
                            nc.tensor.matmul(
                                pt[:, gg * 4 + a, :], w16[:, a, g * P:(g + 1) * P],
                                ident, start=True, stop=True)
                    nc.scalar.copy(
                        wT[:, w * 2:w * 2 + 2, :],
                        pt.rearrange("p (g a) b -> p g (a b)", g=2))
                return wT

            wqT = transpose_weight(wq16, "wqT")
            wkT = transpose_weight(wk16, "wkT")
            wvT = transpose_weight(wv16, "wvT")

            # ---- W_qk = Wk @ Wq^T ;  W_vd = Wv @ Wd2 ----
            wqk = wgt.tile([P, DC, D], F16, tag="wqk")
            wvd = wgt.tile([P, DC, D], F16, tag="wvd")
            for nm, (lhsT, rhs, dst) in {
                "q": (wkT, wqT, wqk), "v": (wvT, wd2, wvd)
            }.items():
                for w in range(2):
                    acc = psk.tile([P, 2, D], F32, tag="pk", name=f"pk_{nm}{w}")
                    for aa in range(2):
                        ach = w * 2 + aa
                        for g in range(DC):
                            nc.tensor.matmul(
                                acc[:, aa, :], lhsT[:, g, ach * P:(ach + 1) * P],
                                rhs[:, g, :], start=(g == 0), stop=(g == DC - 1))
                    nc.scalar.copy(dst[:, w * 2:w * 2 + 2, :], acc)

            # ---- preload variant (for DMA-ablation benchmarking) ----
            pre_hn, pre_en = [], []
            if "dma" in skip:
                for t in range(NT):
                    phn = pre.tile([P, TQ, D], F16, tag=f"phn{t}", name=f"phn{t}")
                    pen = pre.tile([P, TK, D], F16, tag=f"pen{t}", name=f"pen{t}")
                    nc.gpsimd.dma_start(out=phn, in_=h_r[t * P:(t + 1) * P])
                    nc.gpsimd.dma_start(out=pen, in_=e_r[t * P:(t + 1) * P])
                    pre_hn.append(phn)
                    pre_en.append(pen)

            # ================= software-pipelined main loop =================
            def emit_front(t):
                bsl = slice(t * P, (t + 1) * P)
                if "dma" in skip:
                    hn, en = pre_hn[t], pre_en[t]
                else:
                    hn = io.tile([P, TQ, D], F16, tag="hn", name=f"hn{t}")
                    en = io.tile([P, TK, D], F16, tag="en", name=f"en{t}")
                    nc.gpsimd.dma_start(out=hn, in_=h_r[bsl])
                    nc.gpsimd.dma_start(out=en, in_=e_r[bsl])

                hT = work.tile([P, DC, TQ, P], F16, tag="hT", name=f"hT{t}")
                pth = ps.tile([P, 8, P], F32, tag="pt", name=f"pth{t}")
                for i in range(TQ):
                    for c in range(DC):
                        nc.tensor.matmul(
                            pth[:, i * DC + c, :], hn[:, i, c * P:(c + 1) * P],
                            ident, start=True, stop=True)
                nc.scalar.copy(hT, pth.rearrange("p (i c) b -> p c i b", i=TQ))

                eT = work.tile([P, DC, TK, P], F16, tag="eT", name=f"eT{t}")
                for w in range(3):
                    js = (0, 2, 4)[w]
                    jn = (2, 2, 1)[w]
                    pte = ps.tile([P, 8, P], F32, tag="pt", name=f"pte{t}_{w}")
                    for jj in range(jn):
                        for c in range(DC):
                            nc.tensor.matmul(
                                pte[:, jj * DC + c, :],
                                en[:, js + jj, c * P:(c + 1) * P],
                                ident, start=True, stop=True)
                    nc.scalar.copy(
                        eT[:, :, js:js + jn, :],
                        pte[:, 0:jn * DC, :].rearrange("p (j c) b -> p c j b", j=jn))

                kn = work.tile([P, TK, D], F16, tag="kn", name=f"kn{t}")
                for w in range(3):
                    js = (0, 2, 4)[w]
                    jn = (2, 2, 1)[w]
                    acc = psk.tile([P, 2, D], F32, tag="pk", name=f"pkk{t}_{w}")
                    for jj in range(jn):
                        for c in range(DC):
                            nc.tensor.matmul(
                                acc[:, jj, :], eT[:, c, js + jj, :], wqk[:, c, :],
                                start=(c == 0), stop=(c == DC - 1))
                    nc.scalar.copy(kn[:, js:js + jn, :], acc[:, 0:jn, :])

                return dict(t=t, hn=hn, en=en, hT=hT, kn=kn)

            def emit_back(st):
                t, hn, en, hT, kn = st["t"], st["hn"], st["en"], st["hT"], st["kn"]
                bsl = slice(t * P, (t + 1) * P)

                if "attn" in skip:
                    ctx = bwork.tile([P, TQ, D], F16, tag="ctx", name=f"ctx{t}")
                    nc.vector.tensor_copy(ctx, kn[:, 0:2, :])
                else:
                    logits = small.tile([P, TQ, TK], F32, tag="logits", name=f"lg{t}")
                    pdump = small.tile([P, 1], F16, tag="pdump", name=f"pd{t}")
                    for i in range(TQ):
                        for j in range(TK):
                            nc.vector.scalar_tensor_tensor(
                                out=pdump.broadcast_to([P, D]),
                                in0=hn[:, i, :], scalar=1.0, in1=kn[:, j, :],
                                op0=BYP, op1=MUL,
                                accum_out=logits[:, i, j:j + 1])

                    nmx = small.tile([P, TQ], F32, tag="nmx", name=f"nm{t}")
                    pr = small.tile([P, TQ, TK], F32, tag="pr", name=f"pr{t}")
                    sm = small.tile([P, TQ], F32, tag="sm", name=f"sm{t}")
                    rs = small.tile([P, TQ], F32, tag="rs", name=f"rs{t}")
                    attw = small.tile([P, TQ, TK], F32, tag="attw", name=f"at{t}")
                    nc.vector.tensor_reduce(
                        out=nmx, in_=logits, axis=mybir.AxisListType.X,
                        op=mybir.AluOpType.max, negate=True)
                    for i in range(TQ):
                        nc.scalar.activation(
                            out=pr[:, i, :], in_=logits[:, i, :],
                            func=mybir.ActivationFunctionType.Exp,
                            bias=nmx[:, i:i + 1],
                            accum_out=sm[:, i:i + 1])
                    nc.vector.reciprocal(rs, sm)
                    for i in range(TQ):
                        nc.vector.tensor_scalar_mul(attw[:, i, :], pr[:, i, :], rs[:, i:i + 1])

                    ctx = bwork.tile([P, TQ, D], F16, tag="ctx", name=f"ctx{t}")
                    for i in range(TQ):
                        nc.vector.tensor_scalar_mul(ctx[:, i, :], en[:, 0, :], attw[:, i, 0:1])
                        for j in range(1, TK):
                            nc.vector.scalar_tensor_tensor(
                                out=ctx[:, i, :], in0=en[:, j, :],
                                scalar=attw[:, i, j:j + 1], in1=ctx[:, i, :],
                                op0=MUL, op1=ADD)

                cT = bwork.tile([P, DC, TQ, P], F16, tag="cT", name=f"cT{t}")
                ptc = psb.tile([P, 8, P], F32, tag="bk", name=f"ptc{t}")
                for i in range(TQ):
                    for c in range(DC):
                        nc.tensor.matmul(
                            ptc[:, i * DC + c, :], ctx[:, i, c * P:(c + 1) * P],
                            ident, start=True, stop=True)
                nc.vector.tensor_copy(cT, ptc.rearrange("p (i c) b -> p c i b", i=TQ))

                ob = io.tile([P, TQ, D], F32, tag="ob", name=f"ob{t}")
                po = psb.tile([P, TQ, D], F32, tag="bk", name=f"po{t}")
                for i in range(TQ):
                    for c in range(DC):
                        nc.tensor.matmul(po[:, i, :], hT[:, c, i, :], wd1[:, c, :],
                                         start=(c == 0), stop=False)
                    for c in range(DC):
                        nc.tensor.matmul(po[:, i, :], cT[:, c, i, :], wvd[:, c, :],
                                         start=False, stop=(c == DC - 1))
                nc.vector.tensor_copy(ob, po)
                nc.gpsimd.dma_start(out=o_r[bsl], in_=ob)

            loop_cm = tc.For_i(0, reps, 1) if reps > 1 else contextlib.nullcontext()
            with loop_cm:
                pending = {}
                for tt in range(NT + LAG):
                    if tt < NT:
                        pending[tt] = emit_front(tt)
                    if tt >= LAG:
                        emit_back(pending.pop(tt - LAG))

    nc.compile()
    return nc


def kernel(h, enc_out, Wq, Wk, Wv, Wdown, _trace=False):
    h = np.ascontiguousarray(h, dtype=np.float32)
    enc_out = np.ascontiguousarray(enc_out, dtype=np.float32)
    Wq = np.ascontiguousarray(Wq, dtype=np.float32)
    Wk = np.ascontiguousarray(Wk, dtype=np.float32)
    Wv = np.ascontiguousarray(Wv, dtype=np.float32)
    Wdown = np.ascontiguousarray(Wdown, dtype=np.float32)

    if "nc" not in _CACHED:
        _CACHED["nc"] = build()
    nc = _CACHED["nc"]

    h_bm = np.ascontiguousarray(h.transpose(1, 0, 2))        # [B, TQ, D]
    e_bm = np.ascontiguousarray(enc_out.transpose(1, 0, 2))  # [B, TK, D]
    in_maps = []
    for c in range(NCORES):
        sl = slice(c * BL, (c + 1) * BL)
        in_maps.append({
            "h": h_bm[sl],
            "enc": e_bm[sl],
            "Wq": Wq, "Wk": Wk, "Wv": Wv, "Wdown": Wdown,
        })

    res = run_bass_kernel_spmd(nc, in_maps, list(range(NCORES)), trace=_trace)
    out_bm = np.concatenate([r["out"] for r in res.results], axis=0)  # [B, TQ, D]
    out = np.ascontiguousarray(out_bm.transpose(1, 0, 2))
    if _trace:
        kernel.last_result = res
    return out.astype(np.float32)


# revision 9
# speedup vs baseline: 2.0735x; 1.7036x over previous
"""TRN2 Bass kernel for nn_Attention (cross-attention, Tq=2, Tk=5, B=16384, D=512).

Math reformulation (exact):
    logits = h . k~,  k~ = e @ W_qk,  W_qk = Wk @ Wq^T
    att = softmax(logits)
    out = h@Wd1 + ctx@W_vd,   ctx = att @ e,   W_vd = Wv @ Wd2
This removes the q and v projections entirely.

Sharding: pure data parallel over batch, 2048 per core x 8 cores.
Host marshals inputs/outputs to batch-major [B, T, D] for contiguous DMA.
On-chip compute in fp16 (PSUM accumulation fp32); final out fp32.
Main loop is software-pipelined (front: loads/transposes/k~; back: attention/out)
with a lag of 2 batch tiles so PE and DVE streams interleave across tiles.
"""

import contextlib

import numpy as np

import concourse.bass as bass
import concourse.mybir as mybir
import concourse.tile as tile
from concourse import bacc
from concourse.bass_utils import run_bass_kernel_spmd
from concourse.masks import make_identity

F32 = mybir.dt.float32
F16 = mybir.dt.float16
MUL = mybir.AluOpType.mult
ADD = mybir.AluOpType.add
BYP = mybir.AluOpType.bypass

TQ, TK, B, D = 2, 5, 16384, 512
NCORES = 8
BL = B // NCORES          # 2048 batch per core
P = 128                   # partition tile
NT = BL // P              # 16 batch tiles per core
DC = D // P               # 4 contraction chunks
LAG = 3                   # software-pipeline depth (front of t  ||  back of t-LAG)

_CACHED = {}


def build(reps=1, skip=()):
    nc = bacc.Bacc("TRN2", target_bir_lowering=False, debug=False)

    h_d = nc.dram_tensor("h", [BL, TQ, D], F32, kind="ExternalInput")
    e_d = nc.dram_tensor("enc", [BL, TK, D], F32, kind="ExternalInput")
    wq_d = nc.dram_tensor("Wq", [D, D], F32, kind="ExternalInput")
    wk_d = nc.dram_tensor("Wk", [D, D], F32, kind="ExternalInput")
    wv_d = nc.dram_tensor("Wv", [D, D], F32, kind="ExternalInput")
    wd_d = nc.dram_tensor("Wdown", [2 * D, D], F32, kind="ExternalInput")
    o_d = nc.dram_tensor("out", [BL, TQ, D], F32, kind="ExternalOutput")

    h_r = h_d.ap()
    e_r = e_d.ap()
    o_r = o_d.ap()

    with tile.TileContext(nc) as tc:
        with (
            tc.tile_pool(name="wgt", bufs=1) as wgt,
            tc.tile_pool(name="pre", bufs=1) as pre,
            tc.tile_pool(name="io", bufs=LAG + 2) as io,
            tc.tile_pool(name="work", bufs=LAG + 2) as work,
            tc.tile_pool(name="bwork", bufs=2) as bwork,
            tc.tile_pool(name="small", bufs=3) as small,
            tc.tile_pool(name="ps", bufs=2, space="PSUM") as ps,       # "pt": [P,8,P] 2bk x2
            tc.tile_pool(name="psk", bufs=1, space="PSUM") as psk,     # "pk": [P,2,512] 2bk
            tc.tile_pool(name="psb", bufs=1, space="PSUM") as psb,     # "po": 2bk
        ):
            ident = wgt.tile([P, P], F16)
            make_identity(nc, ident)

            # ---- load weights (cast fp32 -> fp16 during DMA) ----
            wq16 = pre.tile([P, DC, D], F16, tag="wq16")
            wk16 = pre.tile([P, DC, D], F16, tag="wk16")
            wv16 = pre.tile([P, DC, D], F16, tag="wv16")
            wd1 = wgt.tile([P, DC, D], F16, tag="wd1")
            wd2 = wgt.tile([P, DC, D], F16, tag="wd2")
            nc.gpsimd.dma_start(out=wq16, in_=wq_d.ap().rearrange("(c p) n -> p c n", p=P))
            nc.gpsimd.dma_start(out=wk16, in_=wk_d.ap().rearrange("(c p) n -> p c n", p=P))
            nc.gpsimd.dma_start(out=wv16, in_=wv_d.ap().rearrange("(c p) n -> p c n", p=P))
            nc.gpsimd.dma_start(out=wd1, in_=wd_d.ap()[:D].rearrange("(c p) n -> p c n", p=P))
            nc.gpsimd.dma_start(out=wd2, in_=wd_d.ap()[D:].rearrange("(c p) n -> p c n", p=P))

            # ---- transpose Wq, Wk, Wv via identity-matmul (2 waves of 8 blocks) ----
            def transpose_weight(w16, name):
                wT = pre.tile([P, DC, D], F16, tag=name, name=name)
                for w in range(2):
                    pt = ps.tile([P, 8, P], F32, tag="pt", name=f"pt_{name}{w}")
                    for gg in range(2):
                        g = w * 2 + gg
                        for a in range(DC):
                            nc.tensor.matmul(
                                pt[:, gg * 4 + a, :], w16[:, a, g * P:(g + 1) * P],
                                ident, start=True, stop=True)
                    nc.scalar.copy(
                        wT[:, w * 2:w * 2 + 2, :],
                        pt.rearrange("p (g a) b -> p g (a b)", g=2))
                return wT

            wqT = transpose_weight(wq16, "wqT")
            wkT = transpose_weight(wk16, "wkT")
            wvT = transpose_weight(wv16, "wvT")

            # ---- W_qk = Wk @ Wq^T ;  W_vd = Wv @ Wd2 ----
            wqk = wgt.tile([P, DC, D], F16, tag="wqk")
            wvd = wgt.tile([P, DC, D], F16, tag="wvd")
            for nm, (lhsT, rhs, dst) in {
                "q": (wkT, wqT, wqk), "v": (wvT, wd2, wvd)
            }.items():
                for w in range(2):
                    acc = psk.tile([P, 2, D], F32, tag="pk", name=f"pk_{nm}{w}")
                    for aa in range(2):
                        ach = w * 2 + aa
                        for g in range(DC):
                            nc.tensor.matmul(
                                acc[:, aa, :], lhsT[:, g, ach * P:(ach + 1) * P],
                                rhs[:, g, :], start=(g == 0), stop=(g == DC - 1))
                    nc.scalar.copy(dst[:, w * 2:w * 2 + 2, :], acc)

            # ---- preload variant (for DMA-ablation benchmarking) ----
            pre_hn, pre_en = [], []
            if "dma" in skip:
                for t in range(NT):
                    phn = pre.tile([P, TQ, D], F16, tag=f"phn{t}", name=f"phn{t}")
                    pen = pre.tile([P, TK, D], F16, tag=f"pen{t}", name=f"pen{t}")
                    nc.gpsimd.dma_start(out=phn, in_=h_r[t * P:(t + 1) * P])
                    nc.gpsimd.dma_start(out=pen, in_=e_r[t * P:(t + 1) * P])
                    pre_hn.append(phn)
                    pre_en.append(pen)

            # ================= software-pipelined main loop =================
            def emit_front(t):
                bsl = slice(t * P, (t + 1) * P)
                if "dma" in skip:
                    hn, en = pre_hn[t], pre_en[t]
                else:
                    hn = io.tile([P, TQ, D], F16, tag="hn", name=f"hn{t}")
                    en = io.tile([P, TK, D], F16, tag="en", name=f"en{t}")
                    nc.gpsimd.dma_start(out=hn, in_=h_r[bsl])
                    nc.gpsimd.dma_start(out=en, in_=e_r[bsl])

                hT = work.tile([P, DC, TQ, P], F16, tag="hT", name=f"hT{t}")
                pth = ps.tile([P, 8, P], F32, tag="pt", name=f"pth{t}")
                for i in range(TQ):
                    for c in range(DC):
                        nc.tensor.matmul(
                            pth[:, i * DC + c, :], hn[:, i, c * P:(c + 1) * P],
                            ident, start=True, stop=True)
                nc.scalar.copy(hT, pth.rearrange("p (i c) b -> p c i b", i=TQ))

                eT = work.tile([P, DC, TK, P], F16, tag="eT", name=f"eT{t}")
                for w in range(3):
                    js = (0, 2, 4)[w]
                    jn = (2, 2, 1)[w]
                    pte = ps.tile([P, 8, P], F32, tag="pt", name=f"pte{t}_{w}")
                    for jj in range(jn):
                        for c in range(DC):
                            nc.tensor.matmul(
                                pte[:, jj * DC + c, :],
                                en[:, js + jj, c * P:(c + 1) * P],
                                ident, start=True, stop=True)
                    nc.scalar.copy(
                        eT[:, :, js:js + jn, :],
                        pte[:, 0:jn * DC, :].rearrange("p (j c) b -> p c j b", j=jn))

                kn = work.tile([P, TK, D], F16, tag="kn", name=f"kn{t}")
                for w in range(3):
                    js = (0, 2, 4)[w]
                    jn = (2, 2, 1)[w]
                    acc = psk.tile([P, 2, D], F32, tag="pk", name=f"pkk{t}_{w}")
                    for jj in range(jn):
                        for c in range(DC):
                            nc.tensor.matmul(
                                acc[:, jj, :], eT[:, c, js + jj, :], wqk[:, c, :],
                                start=(c == 0), stop=(c == DC - 1))
                    nc.scalar.copy(kn[:, js:js + jn, :], acc[:, 0:jn, :])

                return dict(t=t, hn=hn, en=en, hT=hT, kn=kn)

            def emit_back(st):
                t, hn, en, hT, kn = st["t"], st["hn"], st["en"], st["hT"], st["kn"]
                bsl = slice(t * P, (t + 1) * P)

                if "attn" in skip:
                    ctx = bwork.tile([P, TQ, D], F16, tag="ctx", name=f"ctx{t}")
                    nc.vector.tensor_copy(ctx, kn[:, 0:2, :])
                else:
                    logits = small.tile([P, TQ, TK], F32, tag="logits", name=f"lg{t}")
                    pdump = small.tile([P, 1], F16, tag="pdump", name=f"pd{t}")
                    for i in range(TQ):
                        for j in range(TK):
                            nc.vector.scalar_tensor_tensor(
                                out=pdump.broadcast_to([P, D]),
                                in0=hn[:, i, :], scalar=1.0, in1=kn[:, j, :],
                                op0=BYP, op1=MUL,
                                accum_out=logits[:, i, j:j + 1])

                    nmx = small.tile([P, TQ], F32, tag="nmx", name=f"nm{t}")
                    pr = small.tile([P, TQ, TK], F32, tag="pr", name=f"pr{t}")
                    sm = small.tile([P, TQ], F32, tag="sm", name=f"sm{t}")
                    rs = small.tile([P, TQ], F32, tag="rs", name=f"rs{t}")
                    attw = small.tile([P, TQ, TK], F32, tag="attw", name=f"at{t}")
                    nc.vector.tensor_reduce(
                        out=nmx, in_=logits, axis=mybir.AxisListType.X,
                        op=mybir.AluOpType.max, negate=True)
                    for i in range(TQ):
                        nc.scalar.activation(
                            out=pr[:, i, :], in_=logits[:, i, :],
                            func=mybir.ActivationFunctionType.Exp,
                            bias=nmx[:, i:i + 1],
                            accum_out=sm[:, i:i + 1])
                    nc.vector.reciprocal(rs, sm)
                    for i in range(TQ):
                        nc.vector.tensor_scalar_mul(attw[:, i, :], pr[:, i, :], rs[:, i:i + 1])

                    ctx = bwork.tile([P, TQ, D], F16, tag="ctx", name=f"ctx{t}")
                    for i in range(TQ):
                        nc.vector.tensor_scalar_mul(ctx[:, i, :], en[:, 0, :], attw[:, i, 0:1])
                        for j in range(1, TK):
                            nc.vector.scalar_tensor_tensor(
                                out=ctx[:, i, :], in0=en[:, j, :],
                                scalar=attw[:, i, j:j + 1], in1=ctx[:, i, :],
                                op0=MUL, op1=ADD)

                cT = bwork.tile([P, DC, TQ, P], F16, tag="cT", name=f"cT{t}")
                ptc = ps.tile([P, 8, P], F32, tag="pt", name=f"ptc{t}")
                for i in range(TQ):
                    for c in range(DC):
                        nc.tensor.matmul(
                            ptc[:, i * DC + c, :], ctx[:, i, c * P:(c + 1) * P],
                            ident, start=True, stop=True)
                nc.vector.tensor_copy(cT, ptc.rearrange("p (i c) b -> p c i b", i=TQ))

                ob = io.tile([P, TQ, D], F32, tag="ob", name=f"ob{t}")
                po = psb.tile([P, TQ, D], F32, tag="po", name=f"po{t}")
                for i in range(TQ):
                    for c in range(DC):
                        nc.tensor.matmul(po[:, i, :], hT[:, c, i, :], wd1[:, c, :],
                                         start=(c == 0), stop=False)
                    for c in range(DC):
                        nc.tensor.matmul(po[:, i, :], cT[:, c, i, :], wvd[:, c, :],
                                         start=False, stop=(c == DC - 1))
                nc.vector.tensor_copy(ob, po)
                nc.gpsimd.dma_start(out=o_r[bsl], in_=ob)

            loop_cm = tc.For_i(0, reps, 1) if reps > 1 else contextlib.nullcontext()
            with loop_cm:
                pending = {}
                for tt in range(NT + LAG):
                    if tt < NT:
                        pending[tt] = emit_front(tt)
                    if tt >= LAG:
                        emit_back(pending.pop(tt - LAG))

    nc.compile()
    return nc


def kernel(h, enc_out, Wq, Wk, Wv, Wdown, _trace=False):
    h = np.ascontiguousarray(h, dtype=np.float32)
    enc_out = np.ascontiguousarray(enc_out, dtype=np.float32)
    Wq = np.ascontiguousarray(Wq, dtype=np.float32)
    Wk = np.ascontiguousarray(Wk, dtype=np.float32)
    Wv = np.ascontiguousarray(Wv, dtype=np.float32)
    Wdown = np.ascontiguousarray(Wdown, dtype=np.float32)

    if "nc" not in _CACHED:
        _CACHED["nc"] = build()
    nc = _CACHED["nc"]

    h_bm = np.ascontiguousarray(h.transpose(1, 0, 2))        # [B, TQ, D]
    e_bm = np.ascontiguousarray(enc_out.transpose(1, 0, 2))  # [B, TK, D]
    in_maps = []
    for c in range(NCORES):
        sl = slice(c * BL, (c + 1) * BL)
        in_maps.append({
            "h": h_bm[sl],
            "enc": e_bm[sl],
            "Wq": Wq, "Wk": Wk, "Wv": Wv, "Wdown": Wdown,
        })

    res = run_bass_kernel_spmd(nc, in_maps, list(range(NCORES)), trace=_trace)
    out_bm = np.concatenate([r["out"] for r in res.results], axis=0)  # [B, TQ, D]
    out = np.ascontiguousarray(out_bm.transpose(1, 0, 2))
    if _trace:
        kernel.last_result = res
    return out.astype(np.float32)
